# revision 39
# baseline (speedup 1.0000x reference)
# Linear-attention layer (phi = elu+1) on 8 Trainium2 NeuronCores.
#
# Reference computation (per batch b):
#   q = x @ Wq + bq ; k = x @ Wk + bk ; v = x @ Wv + bv      [S, DM] each
#   kv[h] = phi(k_h)^T @ v_h          (sum over ALL of S)    [HD, HD]
#   attn_h = phi(q_h) @ kv[h]                                [S, HD]
#   out = attn @ Wo + bo                                     [S, DM]
#
# Sharding: sequence-parallel. Core c owns S/8 = 512 positions of every
# batch (2048 rows total). kv is a sum over sequence -> each core computes
# a partial kv over its rows, four per-batch 128 KiB AllReduces combine
# them (each fires as soon as its batch finishes phase 1 and hides under
# the q projection), and every core finishes its own rows through attn +
# out_proj. Output rows are disjoint across cores, so no other
# communication is needed.
#
# phi(t) = elu(t) + 1 = exp(min(t, 0)) + relu(t)
#
# Numerics: x/W cast to bf16 on host, matmuls accumulate in fp32 PSUM,
# kv state reduced in bf16 (validated ~3.5e-3 scaled absmax vs fp32 ref).

import numpy as np
import ml_dtypes

B, S, DM, H = 4, 4096, 1024, 16
HD = DM // H          # 64
N_CORES = 8
P = 128
SC = S // N_CORES     # 512 sequence positions per core
R = B * SC            # 2048 rows per core
KC = DM // P          # 8 contraction chunks
NB = SC               # rows per batch on a core (512)
SCB = NB // P         # s-chunks per batch (4)
SCH = R // P          # s-chunks total (16)
NT_R = R // 512       # 512-wide row tiles (4)
ND = DM // 512        # 512-wide feature tiles (2)

_cache = {}


def _build(has_bias):
    import concourse.mybir as mybir
    import concourse.tile as tile
    from concourse import bacc

    fp32 = mybir.dt.float32
    bf16 = mybir.dt.bfloat16
    AF = mybir.ActivationFunctionType
    ALU = mybir.AluOpType

    nc = bacc.Bacc("TRN2", target_bir_lowering=False, debug=False,
                   num_devices=N_CORES)

    x_d = nc.dram_tensor("x", [DM, R], bf16, kind="ExternalInput").ap()
    wq_d = nc.dram_tensor("wq", [DM, DM], bf16, kind="ExternalInput").ap()
    wk_d = nc.dram_tensor("wk", [DM, DM], bf16, kind="ExternalInput").ap()
    wv_d = nc.dram_tensor("wv", [DM, DM], bf16, kind="ExternalInput").ap()
    wo_d = nc.dram_tensor("wo", [DM, DM], bf16, kind="ExternalInput").ap()
    bqc_d = nc.dram_tensor("bqc", [P, KC], fp32, kind="ExternalInput").ap()
    if has_bias:
        bk_d = nc.dram_tensor("bk2", [1, DM], bf16, kind="ExternalInput").ap()
        bv_d = nc.dram_tensor("bv2", [1, DM], bf16, kind="ExternalInput").ap()
        bo_d = nc.dram_tensor("bo2", [1, DM], bf16, kind="ExternalInput").ap()
    out_d = nc.dram_tensor("out", [R, DM], fp32, kind="ExternalOutput").ap()
    with tile.TileContext(nc) as tc:
        with (
            tc.tile_pool(name="big", bufs=1) as big,
            tc.tile_pool(name="stream", bufs=4) as stream,
            tc.tile_pool(name="tmp", bufs=2) as tmpp,
            tc.tile_pool(name="outp", bufs=3) as outp,
            tc.tile_pool(name="psum", bufs=5, space="PSUM") as psum,
            tc.tile_pool(name="kvps", bufs=2, space="PSUM") as kvps,
            tc.tile_pool(name="dram", bufs=2, space="DRAM") as dram,
        ):
            # ---------------- persistent tiles ----------------
            # wk/wv live only through phase 1 (own pool, closed after);
            # attnt is allocated afterwards and reuses their space.
            wkv_pool = tc.tile_pool(name="wkv", bufs=1)
            wkv = wkv_pool.__enter__()
            xt = [big.tile([P, KC, 512], bf16, tag=f"xt{rg}", name=f"xt{rg}")
                  for rg in range(NT_R)]          # x^T, split by row group
            # weights split into 512-wide halves so consumers unblock after
            # 1 MiB instead of 2 MiB of DMA
            wq = [big.tile([P, KC, 512], bf16, tag=f"wq{h}", name=f"wq{h}")
                  for h in range(2)]
            # wk/wv first halves are split into two quarter tiles so the
            # very first projection chains unblock after 0.5MB of DMA
            # (consumer waits are whole-tile, not region)
            wk = [(wkv.tile([P, KC // 2, 512], bf16, tag="wk0a", name="wk0a"),
                   wkv.tile([P, KC // 2, 512], bf16, tag="wk0b", name="wk0b")),
                  wkv.tile([P, KC, 512], bf16, tag="wk1", name="wk1")]
            wv = [(wkv.tile([P, KC // 2, 512], bf16, tag="wv0a", name="wv0a"),
                   wkv.tile([P, KC // 2, 512], bf16, tag="wv0b", name="wv0b")),
                  wkv.tile([P, KC, 512], bf16, tag="wv1", name="wv1")]

            def wpart(w, n, kc):
                if n == 0:
                    return w[0][kc // (KC // 2)][:, kc % (KC // 2), :]
                return w[1][:, kc, :]
            wo = [big.tile([P, KC, 512], bf16, tag=f"wo{h}", name=f"wo{h}")
                  for h in range(2)]
            phiq = big.tile([P, KC, R], bf16, tag="phiq")    # phi(q)^T
            # kv state: head-pair stacked on partitions (even head rows 0:64,
            # odd head rows 64:128); column slot (b*8 + pair)*64
            kv_sb = [big.tile([P, (H // 2) * HD], bf16, tag=f"kv{b}",
                              name=f"kv{b}") for b in range(B)]
            kv_rd = [big.tile([P, (H // 2) * HD], bf16, tag=f"kvr{b}",
                              name=f"kvr{b}") for b in range(B)]
            # block-diag expansion of kv_rd: per (b, pair) a [128,128] block
            # with kv_even at (0:64, 0:64), kv_odd at (64:128, 64:128)
            kv_bd = [big.tile([P, (H // 2) * P], bf16, tag=f"kvbd{b}",
                              name=f"kvbd{b}") for b in range(B)]
            bqc = big.tile([P, KC], fp32, tag="bqc")
            if has_bias:
                bk2 = big.tile([1, DM], bf16, tag="bk2")
                bv2 = big.tile([1, DM], bf16, tag="bv2")
                bo2 = big.tile([1, DM], bf16, tag="bo2")
            ones = big.tile([1, P], bf16, tag="ones")
            zrow = big.tile([1, 512], bf16, tag="zrow")

            KVB = (H // 2) * HD  # 512 columns of kv state per batch
            kv_in = [dram.tile([P, KVB], bf16, tag=f"kvi{b}", name=f"kvi{b}")
                     for b in range(B)]
            kv_out = [dram.tile([P, KVB], bf16, tag=f"kvo{b}", name=f"kvo{b}")
                      for b in range(B)]

            def s512(n):
                return slice(n * 512, (n + 1) * 512)

            # ---------------- loads ----------------
            # x arrives pre-transposed from the host. Everything rides the
            # sync HWDGE queue in consumption order (scalar-queue
            # descriptor gen starves ACT dispatch; SWDGE contends with the
            # gpsimd memsets/collective path). Each load is a single
            # dma_start per tile/region — per-call descriptor generation
            # costs ~0.6-0.8us of sequencer time, so per-128-row-chunk
            # weight loads would serialize ~5us of gen per weight half.
            xt_dr = x_d.rearrange("(c p) r -> p c r", p=P)
            wk_dr = wk_d.rearrange("(c p) n -> p c n", p=P)
            wv_dr = wv_d.rearrange("(c p) n -> p c n", p=P)
            wq_dr = wq_d.rearrange("(c p) n -> p c n", p=P)
            wo_dr = wo_d.rearrange("(c p) n -> p c n", p=P)
            nc.gpsimd.memset(ones[:], 1.0)
            nc.gpsimd.memset(zrow[:], 0.0)
            # xt[0]'s first 128 columns (the g=0 row chunk, 256KB) load
            # ahead of everything: consumer waits are region-level, so the
            # first k-chain (which reads only cols 0:128 of each kc plane)
            # unblocks right as the PE warmup matmuls finish (~11.5us),
            # instead of waiting ~18us for the whole 1MB tile.
            nc.sync.dma_start(xt[0][:, :, 0:P], xt_dr[:, :, 0:P])
            nc.sync.dma_start(wk[0][0][:], wk_dr[:, 0:4, s512(0)])
            nc.sync.dma_start(xt[0][:, :, P:512], xt_dr[:, :, P:512])
            # wv's first half rides the scalar HWDGE queue: its two
            # descriptor gens finish long before the first real ACT op
            # (~18us), and the parallel ring lands wv ~7us earlier than
            # queueing it behind wk/x on sync — the early v-chain stalls
            # (measured 2.0-2.4us each) disappear. Later weights stay off
            # the scalar queue so phase-1 ACT dispatch is never starved.
            nc.scalar.dma_start(wv[0][0][:], wv_dr[:, 0:4, s512(0)])
            nc.sync.dma_start(wk[0][1][:], wk_dr[:, 4:8, s512(0)])
            nc.scalar.dma_start(wv[0][1][:], wv_dr[:, 4:8, s512(0)])
            for w_sb, w_dr in ((wk, wk_dr), (wv, wv_dr)):
                nc.sync.dma_start(w_sb[1][:], w_dr[:, :, s512(1)])
            for rg in range(1, NT_R):
                nc.sync.dma_start(xt[rg][:], xt_dr[:, :, s512(rg)])
            for w_sb, w_dr in ((wq, wq_dr), (wo, wo_dr)):
                for h in range(2):
                    nc.sync.dma_start(w_sb[h][:], w_dr[:, :, s512(h)])
            nc.gpsimd.dma_start(bqc[:], bqc_d)
            if has_bias:
                nc.gpsimd.dma_start(bk2[:], bk_d)
                nc.gpsimd.dma_start(bv2[:], bv_d)
                nc.gpsimd.dma_start(bo2[:], bo_d)
            # warm the ACT Exp/Relu LUTs during the launch/DMA window so the
            # first real phi ops skip the cold table load (~2us)
            wtile = big.tile([1, 8], bf16, tag="warm")
            nc.scalar.activation(out=wtile[:], in_=zrow[0:1, 0:8], func=AF.Exp)
            nc.scalar.activation(out=wtile[:], in_=zrow[0:1, 0:8], func=AF.Relu)
            # Warm the PE's HAM clock gate while the first x/w DMAs are in
            # flight: the activity monitor needs ~3.4us of sustained matmul
            # work before it lifts the 1.2GHz cold throttle, and the window
            # is free-running — burning it on dummy matmuls (only ones/zrow
            # needed, available at ~6.5us) means the first real projection
            # matmuls at ~10.5us issue at full clock instead of paying the
            # ramp themselves.
            warmps = psum.tile([P, 512], fp32, tag="pp")
            for _ in range(7):
                nc.tensor.matmul(warmps[:], lhsT=ones[:], rhs=zrow[:],
                                 start=True, stop=True)
            nc.vector.tensor_copy(out=wtile[:], in_=warmps[0:1, 0:8])
            for b in range(B):
                nc.gpsimd.memset(kv_bd[b][:], 0.0)

            # ---------- phase 1: k/v projections + phi(k) + partial kv ----------
            # The kv slot matmuls for chunk i are emitted after chunk i+1's
            # projection matmuls: the PE then never waits on the phi chain
            # (DVE/ACT) that produces kch/vch — at kernel start that
            # pipeline latency would otherwise idle the PE ~1.7us.
            def emit_slots(kvp, sc, kch, vch):
                for pr in range(H // 2):
                    j, col = pr // 4, (pr % 4) * P
                    # full pair x pair cross-product; diagonal 64x64
                    # blocks are the two heads' kv states. The first
                    # slot matmul of sc=0 carries start=True: it clears
                    # the whole bank's has_written bits, so the other
                    # three slots' first writes (start=False, bits
                    # clear) overwrite rather than accumulate — no
                    # separate bank-zeroing matmul needed.
                    nc.tensor.matmul(
                        kvp[j][:, col:col + P],
                        lhsT=kch[:, pr * P:(pr + 1) * P],
                        rhs=vch[:, pr * P:(pr + 1) * P],
                        start=(sc == 0 and pr % 4 == 0),
                        stop=(sc == SCB - 1 and pr % 4 == 3),
                        skip_group_check=True)

            def emit_extract(b, kvp):
                for h in range(H):
                    pr = h // 2
                    j, col = pr // 4, (pr % 4) * P + (h % 2) * HD
                    rows = slice((h % 2) * HD, (h % 2 + 1) * HD)
                    slot = pr * HD
                    nc.vector.tensor_copy(
                        out=kv_sb[b][rows, slot:slot + HD],
                        in_=kvp[j][rows, col:col + HD])

            pending = None    # (b, kvp, sc, kch, vch) awaiting slot matmuls
            kvp_of = {}
            for g in range(B * SCB):
                b, sc = g // SCB, g % SCB
                if sc == 0:
                    kvp_of[b] = [kvps.tile([P, 512], fp32, tag="kvp0",
                                           name="kvp0", bufs=2),
                                 kvps.tile([P, 512], fp32, tag="kvp1",
                                           name="kvp1", bufs=1)]
                kch = stream.tile([P, DM], bf16, tag="kch")
                vch = stream.tile([P, DM], bf16, tag="vch")
                for n in range(ND):
                    kps = psum.tile([P, 512], fp32, tag="pp")
                    vps = psum.tile([P, 512], fp32, tag="pp")
                    for kc in range(KC):
                        nc.tensor.matmul(
                            kps[:],
                            lhsT=xt[g // 4][:, kc,
                                            (g % 4) * P:(g % 4 + 1) * P],
                            rhs=wpart(wk, n, kc),
                            start=(kc == 0),
                            stop=(not has_bias and kc == KC - 1))
                    if has_bias:
                        nc.tensor.matmul(kps[:], lhsT=ones[:],
                                         rhs=bk2[:, s512(n)],
                                         start=False, stop=True)
                    for kc in range(KC):
                        nc.tensor.matmul(
                            vps[:],
                            lhsT=xt[g // 4][:, kc,
                                            (g % 4) * P:(g % 4 + 1) * P],
                            rhs=wpart(wv, n, kc),
                            start=(kc == 0),
                            stop=(not has_bias and kc == KC - 1))
                    if has_bias:
                        nc.tensor.matmul(vps[:], lhsT=ones[:],
                                         rhs=bv2[:, s512(n)],
                                         start=False, stop=True)
                    # phi(k) = exp(min(k,0)) + relu(k)
                    ut = tmpp.tile([P, 512], bf16, tag="u")
                    nc.vector.tensor_scalar_min(out=ut[:], in0=kps[:],
                                                scalar1=0.0)
                    rt = tmpp.tile([P, 512], bf16, tag="r")
                    nc.scalar.activation(out=rt[:], in_=kps[:], func=AF.Relu)
                    nc.vector.tensor_copy(out=vch[:, s512(n)], in_=vps[:])
                    et = tmpp.tile([P, 512], bf16, tag="e")
                    nc.scalar.activation(out=et[:], in_=ut[:], func=AF.Exp)
                    nc.vector.tensor_add(out=kch[:, s512(n)], in0=et[:],
                                         in1=rt[:])
                if pending is not None:
                    pb, pkvp, psc, pk, pv = pending
                    emit_slots(pkvp, psc, pk, pv)
                    if psc == SCB - 1:
                        emit_extract(pb, pkvp)
                pending = (b, kvp_of[b], sc, kch, vch)
            pb, pkvp, psc, pk, pv = pending
            emit_slots(pkvp, psc, pk, pv)
            emit_extract(pb, pkvp)
            # wk/wv dead from here; free their SBUF for attnt
            wkv_pool.__exit__(None, None, None)
            attnt = big.tile([P, KC, R], bf16, tag="attnt")  # attn^T


            # ---------- phase 3: q^T projection + phi ----------
            for m in range(KC):
                for nt in range(NT_R):
                    qps = psum.tile([P, 512], fp32, tag="pp")
                    for kc in range(KC):
                        nc.tensor.matmul(
                            qps[:],
                            lhsT=wq[m // 4][:, kc, (m % 4) * P:(m % 4 + 1) * P],
                            rhs=xt[nt][:, kc, :],
                            start=(kc == 0), stop=(kc == KC - 1))
                    ut = tmpp.tile([P, 512], bf16, tag="u")
                    nc.vector.tensor_scalar(out=ut[:], in0=qps[:],
                                            scalar1=bqc[:, m:m + 1],
                                            scalar2=0.0,
                                            op0=ALU.add, op1=ALU.min)
                    rt = tmpp.tile([P, 512], bf16, tag="r")
                    nc.scalar.activation(out=rt[:], in_=qps[:], func=AF.Relu,
                                         bias=bqc[:, m:m + 1], scale=1.0)
                    et = tmpp.tile([P, 512], bf16, tag="e")
                    nc.scalar.activation(out=et[:], in_=ut[:], func=AF.Exp)
                    nc.vector.tensor_add(out=phiq[:, m, s512(nt)], in0=et[:],
                                         in1=rt[:])

            # per-batch AllReduces: each fires as soon as that batch's
            # partial kv is ready (data deps gate execution, not emission
            # order), hiding rendezvous skew under remaining compute. A
            # single end-of-phase-1 AllReduce measures ~35us wall (12us
            # rendezvous + mesh) and lands AFTER phase 3 ends -> 20us of PE
            # idle at the phase-4 boundary. Keep them per-batch.
            # All four bounce-DMAs + triggers first, then all readbacks:
            # the AllReduce completion wait rides the readback DMA, so this
            # ordering lets AR[b+1] trigger without waiting for AR[b] to
            # finish — the four collectives pipeline on the TOPSP instead
            # of compounding a cross-core rendezvous per batch.
            for b in range(B):
                nc.gpsimd.dma_start(kv_in[b][:], kv_sb[b][:])
                nc.gpsimd.collective_compute(
                    "AllReduce",
                    mybir.AluOpType.add,
                    replica_groups=[list(range(N_CORES))],
                    ins=[kv_in[b].opt()],
                    outs=[kv_out[b].opt()],
                )
            for b in range(B):
                nc.gpsimd.dma_start(kv_rd[b][:], kv_out[b][:])

            # ---------- phase 4: attn^T = kv^T @ phi(q)^T per (b, pair) ----------
            # kv_bd (block-diag expansion of kv_rd) is built by SBUF->SBUF
            # DMAs on the gpsimd queue, NOT compute-engine copies: any
            # AR-gated op placed on DVE/ACT can be woven by the Tile
            # scheduler (whose collective cost model is near-zero) into a
            # modeled-idle slot in the middle of phase 1's FIFO, where its
            # semaphore wait head-of-line-blocks the engine for the real
            # ~100us AllReduce latency. The gpsimd queue already serializes
            # behind the AR readbacks, so these DMAs gate nothing else.
            for b in range(B):
                for half in range(2):
                    rows = slice(half * HD, (half + 1) * HD)
                    src = kv_rd[b][rows].rearrange("p (r e) -> p r e", e=HD)
                    dst = kv_bd[b][rows].rearrange(
                        "p (r e) -> p r e", e=P)[:, :, half * HD:(half + 1) * HD]
                    nc.gpsimd.dma_start(dst, src)
            # Phase 5 is interleaved per batch: batch b's out-projection
            # (17us of PE work) is emitted right after its attn matmuls,
            # so batches 0..2's out-proj covers a late AllReduce for batch
            # 3 — cross-core rendezvous jitter (measured up to ~25us) then
            # stalls nothing instead of idling the PE at a phase boundary.
            for b in range(B):
                for pr in range(H // 2):
                    ap = psum.tile([P, NB], fp32, tag="pp")
                    bds = pr * P
                    nc.tensor.matmul(
                        ap[:],
                        lhsT=kv_bd[b][:, bds:bds + P],
                        rhs=phiq[:, pr, b * NB:(b + 1) * NB],
                        start=True, stop=True)
                    if pr % 2:
                        nc.vector.tensor_copy(
                            out=attnt[:, pr, b * NB:(b + 1) * NB], in_=ap[:])
                    else:
                        nc.scalar.activation(
                            out=attnt[:, pr, b * NB:(b + 1) * NB],
                            in_=ap[:], func=AF.Copy)
                # ---------- phase 5: out = attn @ Wo + bo (batch b) ----------
                for g in range(b * SCB, (b + 1) * SCB):
                    for n in range(ND):
                        ops = psum.tile([P, 512], fp32, tag="pp")
                        for kc in range(KC):
                            nc.tensor.matmul(
                                ops[:], lhsT=attnt[:, kc, g * P:(g + 1) * P],
                                rhs=wo[n][:, kc, :],
                                start=(kc == 0),
                                stop=(not has_bias and kc == KC - 1))
                        if has_bias:
                            nc.tensor.matmul(ops[:], lhsT=ones[:],
                                             rhs=bo2[:, s512(n)],
                                             start=False, stop=True)
                        osb = outp.tile([P, 512], fp32, tag="osb")
                        if g == SCH - 1 and n == ND - 1:
                            nc.vector.tensor_copy(out=osb[:], in_=ops[:])
                        else:
                            nc.scalar.activation(out=osb[:], in_=ops[:],
                                                 func=AF.Copy)
                        nc.sync.dma_start(out_d[g * P:(g + 1) * P, s512(n)],
                                          osb[:])

    nc.compile()
    return nc


def _get_nc(has_bias):
    key = ("nc", has_bias)
    if key not in _cache:
        _cache[key] = _build(has_bias)
    return _cache[key]


def _has_bias(inputs):
    return any(np.any(np.asarray(inputs[k], np.float32))
               for k in ("bk", "bv", "bo"))


def _make_in_maps(inputs, has_bias):
    bf16 = ml_dtypes.bfloat16
    x = np.asarray(inputs["x"], dtype=np.float32)
    ws = {k: np.ascontiguousarray(np.asarray(inputs[k], np.float32).astype(bf16))
          for k in ("Wq", "Wk", "Wv", "Wo")}
    bq = np.asarray(inputs["bq"], np.float32)
    bqc = np.ascontiguousarray(bq.reshape(KC, P).T.astype(np.float32))
    brow = {k: np.ascontiguousarray(
                np.asarray(inputs[k], np.float32).astype(bf16).reshape(1, DM))
            for k in ("bk", "bv", "bo")}
    xb = x.astype(bf16)
    in_maps = []
    for c in range(N_CORES):
        xs = np.ascontiguousarray(
            xb[:, c * SC:(c + 1) * SC, :].reshape(R, DM).T)
        m = {
            "x": xs,
            "wq": ws["Wq"], "wk": ws["Wk"], "wv": ws["Wv"], "wo": ws["Wo"],
            "bqc": bqc,
        }
        if has_bias:
            m.update({"bk2": brow["bk"], "bv2": brow["bv"],
                      "bo2": brow["bo"]})
        in_maps.append(m)
    return in_maps


def _run(inputs, **kw):
    from concourse import bass_utils
    hb = _has_bias(inputs)
    nc = _get_nc(hb)
    in_maps = _make_in_maps(inputs, hb)
    res = bass_utils.run_bass_kernel_spmd(
        nc, in_maps, core_ids=list(range(N_CORES)), **kw)
    out = np.empty((B, S, DM), np.float32)
    for c in range(N_CORES):
        out[:, c * SC:(c + 1) * SC, :] = res.results[c]["out"].reshape(B, SC, DM)
    return out, res


def kernel(**inputs) -> np.ndarray:
    out, _ = _run(inputs)
    return out

